# revision 36
# baseline (speedup 1.0000x reference)
"""Trainium2 Bass kernel for nn_BarrierPolicy (CBF-QP safety filter).

Data-parallel over batch: 8 cores x 32768 samples, processed in 4 groups of
8192 samples (4 xview tiles of 2048).

Phase A (per group): DMA x tiles, PE-transpose to SP2 layout, then the MLP
with weight-major matmul runs (one Ldweights per layer): L1 uses a single
(16,128) weight against rhs partition slices (K=16), L2 a single block-pair
weight, L3 a single fused (128,18) weight producing px and the alpha logit
together; dynamics matmuls (A x, -2 G^T x) and PE-transposes back to xview.
Activation-engine evacuations are fused into (128,1024) ops.

Phase B (per group, overlaps the next group's Phase A): optimistic-slope
Newton solve of the per-sample box-QP dual in sign-transformed space:
  ur = lam*gt - pt ; uc = clip(ur) ; c = c0 + sum(gt*uc)
  S  = sum(q * [ur < 1])   (upper bound on all future slopes -> monotone
                            convergence from below; all-saturated infeasible
                            rows diverge to huge lam = reference saturation)
  lam = max(lam - c/S, 0)
c0 and a 1e-12 epsilon ride in a 9th reduction lane. Final u = clip(lam*g-p).

Layouts (per tile of 2048 samples):
  xview: SBUF (128, 128): partition r, col 16b+8s0+j <-> sample 256b+2r+s0,
  coord j; slot: per-sample scalars (128, 16): partition r, col 2b+s0.
"""
import numpy as np

B_FULL, N = 262144, 8
NCORES = 8
S = B_FULL // NCORES          # 32768 samples per core
TILE = 2048
NT = S // TILE                # 16 tiles
NSLOT = S // 128              # 256 slot cols per core
import os
GROUPS = eval(os.environ.get('KGROUPS', '[2, 4, 4, 4, 2]'))
NLANES = int(os.environ.get('KLANES', '1'))
NG = len(GROUPS)              # group for early B start, small last for tail
GBASE = [sum(GROUPS[:i]) for i in range(NG)]
GROUP = 4                     # max group size (psum/scratch tile shapes)
FCG = 128 * GROUP
SLG = 16 * GROUP
T_NEWTON = 5
EPS = 1e-12

_CACHE = {}

_CSHAPES = dict(TL2=(128, 128), TL3F=(128, 18),
                TDA=(128, 128), TDG=(128, 128), ID128=(128, 128), IDr=(128, 128),
                B1v=(128, 1), B2v=(128, 1), B31x=(128, 1), B32e=(128, 1),
                **{f"TL1E{b}": (128, 128) for b in range(8)})
_RKEYS = ["TL2", "TDA", "TDG", "IDr"] + [f"TL1E{b}" for b in range(8)]
_FKEYS = ["TL3F", "ID128", "B1v", "B2v", "B31x", "B32e"]
_RW = sum(_CSHAPES[k][1] for k in _RKEYS)
_FW = sum(_CSHAPES[k][1] for k in _FKEYS)


def _consts(W1, b1, W21, b21, W22, b22, W31, b31, W32, b32, A, G):
    f32 = np.float32
    out = {}
    for b in range(8):
        T = np.zeros((128, 128), f32)
        for s0 in range(2):
            T[16 * b + 8 * s0:16 * b + 8 * s0 + 8, 64 * s0:64 * s0 + 64] = W1
        out[f"TL1E{b}"] = T
    TL2 = np.zeros((128, 128), f32)
    for s0 in range(2):
        TL2[64 * s0:64 * s0 + 64, 32 * s0:32 * s0 + 32] = W21
        TL2[64 * s0:64 * s0 + 64, 64 + 32 * s0:64 + 32 * s0 + 32] = W22
    TL3F = np.zeros((128, 18), f32)        # fused px + alpha-logit head
    for s0 in range(2):
        TL3F[32 * s0:32 * s0 + 32, 8 * s0:8 * s0 + 8] = W31
        TL3F[64 + 32 * s0:96 + 32 * s0, 16 + s0:17 + s0] = W32
    TDA = np.kron(np.eye(16, dtype=f32), A.T.astype(f32))         # out = A x
    TDG = np.kron(np.eye(16, dtype=f32), (-2.0 * G).astype(f32))  # out = -2 G^T x
    ID128 = np.eye(128, dtype=f32)
    B1v = np.concatenate([b1, b1]).reshape(128, 1).astype(f32)
    B2v = np.concatenate([b21, b21, b22, b22]).reshape(128, 1).astype(f32)
    B31x = np.zeros((128, 1), f32)         # px bias rows; alpha rows stay 0
    for m in range(4):
        for s0 in range(2):
            B31x[32 * m + 8 * s0:32 * m + 8 * s0 + 8, 0] = b31
    B32e = np.full((128, 1), float(b32[0]), f32)
    out.update(TL2=TL2, TL3F=TL3F, TDA=TDA, TDG=TDG, ID128=ID128, IDr=ID128,
               B1v=B1v, B2v=B2v, B31x=B31x, B32e=B32e)
    return out


def build_kernel(nc, tc, x_d, u_d, cds):
    from concourse import mybir
    f32 = mybir.dt.float32
    f32r = mybir.dt.float32r
    bf16 = mybir.dt.bfloat16
    AL = mybir.AluOpType
    AF = mybir.ActivationFunctionType
    XL = mybir.AxisListType.X

    with (
        tc.tile_pool(name="const", bufs=1) as cpool,
        tc.tile_pool(name="pers", bufs=1) as pers,
        tc.tile_pool(name="work", bufs=2) as work,
        tc.tile_pool(name="psT", bufs=2, space="PSUM") as psT,
        tc.tile_pool(name="psW", bufs=2, space="PSUM") as psW,
        tc.tile_pool(name="psL", bufs=1, space="PSUM") as psL,
    ):
        CBR = cpool.tile([128, _RW], f32r, tag="CBR", name="CBR")
        CBF = cpool.tile([128, _FW], f32, tag="CBF", name="CBF")
        nc.sync.dma_start(CBR[:], cds["CBLKr"][:])
        nc.sync.dma_start(CBF[:], cds["CBLKf"][:])
        C, off = {}, 0
        for k in _RKEYS:
            w = _CSHAPES[k][1]
            C[k] = CBR[:, off:off + w]
            off += w
        off = 0
        for k in _FKEYS:
            w = _CSHAPES[k][1]
            C[k] = CBF[:, off:off + w]
            off += w

        FC = S // 16   # 2048 xview cols per core
        def fc_tile(tag):
            return pers.tile([128, FC], f32, tag=tag, name=tag)
        x_xv, p_xv, g_xv = fc_tile("x_xv"), fc_tile("p_xv"), fc_tile("g_xv")
        gt_xv, q_xv = fc_tile("gt_xv"), fc_tile("q_xv")
        pt_xv = pers.tile([128, FC], bf16, tag="pt_xv", name="pt_xv")
        def sl_tile(tag, mult=1):
            return pers.tile([128, NSLOT * mult], f32, tag=tag, name=tag)
        alpha4, lfhx, sxx = sl_tile("alpha4"), sl_tile("lfhx"), sl_tile("sxx")
        lams, ccs, sss = sl_tile("lams"), sl_tile("ccs"), sl_tile("sss")
        rs, t1s, t2s = sl_tile("rs"), sl_tile("t1s"), sl_tile("t2s")
        prod9, qm9 = sl_tile("prod9", 9), sl_tile("qm9", 9)
        nc.gpsimd.memset(
            qm9[:].rearrange("p (c j) -> p c j", j=9)[:, :, 8:9], EPS)

        x3 = lambda ap: ap.rearrange("p (c j) -> p c j", j=8)
        x9 = lambda ap: ap.rearrange("p (c j) -> p c j", j=9)
        V, GP, SC = nc.vector, nc.gpsimd, nc.scalar

        def phase_a_group(g):
            nt = GROUPS[g]
            tb = GBASE[g]
            fcg, slg = 128 * nt, 16 * nt
            csg = slice(128 * tb, 128 * (tb + nt))
            ssg = slice(16 * tb, 16 * (tb + nt))
            # ---- load + transpose to SP2 ----
            nc.sync.dma_start(
                x_xv[:, csg].rearrange("p (c j) -> p c j", j=8),
                x_d[:].rearrange("(p c) j -> p c j", p=128)[:, ssg, :])
            xTP = psT.tile([128, 4, 128], f32, tag="tp", name="xTP")
            for tt in range(nt):
                t = tb + tt
                cs = slice(128 * t, 128 * t + 128)
                nc.tensor.transpose(xTP[:, tt, :], x_xv[:, cs], C["ID128"])
            xsp2g = work.tile([128, 4, 128], f32r, tag="xsp2g", name="xsp2g")
            SC.activation(xsp2g[:, 0:nt, :], xTP[:, 0:nt, :], AF.Copy)

            # ---- L1 (weight-major, free=512) ----
            h1 = work.tile([128, 8, 512], f32r, tag="h1", name="h1", bufs=1)
            for pair in range(4):
                h1P = psW.tile([128, 2, 512], f32, tag="mmP", name="h1P")
                for s in range(2):
                    b = 2 * pair + s
                    nc.tensor.matmul(h1P[:, s, 0:fcg], C[f"TL1E{b}"],
                                     xsp2g[:, 0:nt, :])
                SC.activation(h1[:, 2 * pair:2 * pair + 2, 0:fcg],
                              h1P[:, :, 0:fcg], AF.Relu, bias=C["B1v"])

            # ---- L2 (single weight) ----
            x2 = work.tile([128, 8, 512], f32, tag="x2", name="x2", bufs=1)
            for pair in range(4):
                x2P = psW.tile([128, 2, 512], f32, tag="mmP", name="x2P")
                for s in range(2):
                    b = 2 * pair + s
                    nc.tensor.matmul(x2P[:, s, 0:fcg], C["TL2"],
                                     h1[:, b, 0:fcg])
                SC.activation(x2[:, 2 * pair:2 * pair + 2, 0:fcg],
                              x2P[:, :, 0:fcg], AF.Relu, bias=C["B2v"])

            # ---- L3 fused px+alpha (single (128,18) weight) ----
            pxale = work.tile([128, 3, 512], f32r, tag="pxale", name="pxale",
                              bufs=1)
            for hf in range((fcg + 255) // 256):
                w = min(256, fcg - 256 * hf)
                fs = slice(256 * hf, 256 * hf + w)
                pxalP = psL.tile([128, 3, 256], f32, tag="pxalP",
                                 name=f"pxalP{hf}")
                for b in range(8):
                    m3, k3 = b % 3, b // 3
                    nc.tensor.matmul(pxalP[32 * m3:32 * m3 + 18, k3, 0:w],
                                     C["TL3F"], x2[:, b, fs])
                for m3 in range(3):
                    kk = 3 if m3 < 2 else 2
                    SC.activation(pxale[32 * m3:32 * m3 + 18, 0:kk, fs],
                                  pxalP[32 * m3:32 * m3 + 18, 0:kk, 0:w],
                                  AF.Identity,
                                  bias=C["B31x"][32 * m3:32 * m3 + 18, :])

            # ---- dynamics ----
            dynA = psT.tile([128, 4, 128], f32, tag="tp", name="dynA")
            nc.tensor.matmul(dynA[:, 0:nt, :], C["TDA"], xsp2g[:, 0:nt, :])
            axsg = work.tile([128, 4, 128], f32r, tag="axsg", name="axsg")
            SC.activation(axsg[:, 0:nt, :], dynA[:, 0:nt, :], AF.Copy)
            dynG = psT.tile([128, 4, 128], f32, tag="tp", name="dynG")
            nc.tensor.matmul(dynG[:, 0:nt, :], C["TDG"], xsp2g[:, 0:nt, :])
            gsp2g = work.tile([128, 4, 128], f32r, tag="gsp2g", name="gsp2g")
            SC.activation(gsp2g[:, 0:nt, :], dynG[:, 0:nt, :], AF.Copy)

            # ---- transposes back to xview ----
            gT = psT.tile([128, 4, 128], f32r, tag="tp", name="gT")
            for tt in range(nt):
                nc.tensor.transpose(gT[:, tt, :], gsp2g[:, tt, :], C["IDr"])
            SC.activation(g_xv[:, csg].rearrange("p (a b) -> p a b", a=nt),
                          gT[:, 0:nt, :], AF.Copy)
            aT = psT.tile([128, 4, 128], f32r, tag="tp", name="aT")
            for tt in range(nt):
                nc.tensor.transpose(aT[:, tt, :], axsg[:, tt, :], C["IDr"])

            # ---- barrier scalars: Lfhx, ||x||^2 ----
            prodA = work.tile([128, 4, 128], f32, tag="prodA", name="prodA")
            V.scalar_tensor_tensor(
                prodA[:, 0:nt, :], aT[:, 0:nt, :], -2.0,
                x_xv[:, csg].rearrange("p (a b) -> p a b", a=nt),
                AL.mult, AL.mult)
            V.tensor_reduce(lfhx[:, ssg],
                            prodA[:, 0:nt, :].rearrange(
                                "p a (c j) -> p (a c) j", j=8),
                            XL, AL.add)
            sqxg = work.tile([128, 512], f32, tag="sqxg", name="sqxg")
            GP.tensor_tensor(sqxg[:, 0:fcg], x_xv[:, csg], x_xv[:, csg],
                             AL.mult)
            V.tensor_reduce(sxx[:, ssg], x3(sqxg[:, 0:fcg]), XL, AL.add)

            # ---- px/alpha transposes + extraction ----
            for k3 in range(3):
                nm = 3 if k3 < 2 else 2
                pT = psT.tile([128, 4, 128], f32r, tag="tp", name=f"pT{k3}")
                for tt in range(nt):
                    nc.tensor.transpose(pT[:, tt, :],
                                        pxale[:, k3, 128 * tt:128 * tt + 128],
                                        C["IDr"])
                src = pT[:, 0:nt, :].rearrange("p t (m h x) -> p t m h x",
                                               m=4, h=2, x=16)
                dstp = p_xv[:, csg].rearrange(
                    "p (t b x) -> p t b x", t=nt, b=8,
                    x=16)[:, :, 3 * k3:3 * k3 + nm]
                V.tensor_copy(dstp, src[:, :, 0:nm, 0, :])
                dsta = alpha4[:, ssg].rearrange(
                    "p (t b s) -> p t b s", t=nt, b=8,
                    s=2)[:, :, 3 * k3:3 * k3 + nm]
                V.tensor_copy(dsta, src[:, :, 0:nm, 1, 0:2])
            SC.activation(alpha4[:, ssg], alpha4[:, ssg], AF.Sigmoid,
                          bias=C["B32e"])

        def phase_b_group(g, nlanes=NLANES):
            nt = GROUPS[g]
            tb = GBASE[g]
            fcg, slg = 128 * nt // nlanes, 16 * nt // nlanes
            lanes = []
            for h in range(nlanes):
                cs = slice(128 * tb + fcg * h, 128 * tb + fcg * (h + 1))
                ss = slice(16 * tb + slg * h, 16 * tb + slg * (h + 1))
                s9 = slice(9 * (16 * tb + slg * h),
                           9 * (16 * tb + slg * (h + 1)))
                L = dict(
                    gt=gt_xv[:, cs], pt=pt_xv[:, cs], q=q_xv[:, cs],
                    p=p_xv[:, cs], gg=g_xv[:, cs],
                    lam=lams[:, ss], cc=ccs[:, ss], svs=sss[:, ss],
                    r=rs[:, ss], d1=t1s[:, ss], d2=t2s[:, ss],
                    p9=x9(prod9[:, s9]), q9=x9(qm9[:, s9]),
                    sxx=sxx[:, ss], alpha4=alpha4[:, ss], lfhx=lfhx[:, ss],
                    ss=ss,
                    sgx=work.tile([128, fcg], f32, tag=f"sgx{h}", name="sgx"),
                    ur=work.tile([128, fcg], bf16, tag=f"ur{h}", name="ur"),
                    uc=work.tile([128, fcg], bf16, tag=f"uc{h}", name="uc"),
                    mt=work.tile([128, fcg], bf16, tag=f"mt{h}", name="mt"),
                )
                lanes.append(L)
            bc = lambda ap: ap.broadcast_to((128, slg, 8))

            # preamble: transform + c0 (c0 lands in prod9's 9th lane)
            for L in lanes:
                SC.sign(L["sgx"][:], L["gg"])
            for L in lanes:
                SC.activation(L["gt"], L["gg"], AF.Abs)
            for L in lanes:
                GP.tensor_tensor(L["q"], L["gg"], L["gg"], AL.mult)
            for L in lanes:
                V.tensor_tensor(L["pt"], L["sgx"][:], L["p"], AL.mult)
            for L in lanes:
                GP.tensor_scalar(L["d1"], L["sxx"], -4.0, 64.0, AL.mult, AL.add)
            for L in lanes:
                GP.tensor_tensor(L["d2"], L["alpha4"], L["d1"], AL.mult)
            for L in lanes:
                GP.tensor_tensor(L["p9"][:, :, 8], L["d2"], L["lfhx"], AL.add)

            for it in range(T_NEWTON):
                if it == 0:
                    # lam = 0: ur = -pt
                    for L in lanes:
                        V.tensor_scalar(L["uc"][:], L["pt"], -1.0, 1.0,
                                        AL.mult, AL.min)
                    for L in lanes:
                        V.tensor_scalar(L["uc"][:], L["uc"][:], -1.0, None,
                                        AL.max)
                    for L in lanes:
                        GP.tensor_tensor(L["p9"][:, :, 0:8], x3(L["gt"]),
                                         x3(L["uc"][:]), AL.mult)
                    for L in lanes:
                        V.tensor_scalar(L["mt"][:], L["pt"], -1.0, None,
                                        AL.is_gt)
                else:
                    for L in lanes:
                        GP.tensor_tensor(x3(L["ur"][:]), bc(L["lam"]),
                                         x3(L["gt"]), AL.mult)
                    for L in lanes:
                        V.tensor_tensor(L["ur"][:], L["ur"][:], L["pt"],
                                        AL.subtract)
                    for L in lanes:
                        V.tensor_scalar(L["uc"][:], L["ur"][:], 1.0, -1.0,
                                        AL.min, AL.max)
                    for L in lanes:
                        GP.tensor_tensor(L["p9"][:, :, 0:8], x3(L["gt"]),
                                         x3(L["uc"][:]), AL.mult)
                    for L in lanes:
                        V.tensor_scalar(L["mt"][:], L["ur"][:], 1.0, None,
                                        AL.is_lt)
                for L in lanes:
                    GP.tensor_tensor(L["q9"][:, :, 0:8], x3(L["q"]),
                                     x3(L["mt"][:]), AL.mult)
                for L in lanes:
                    V.tensor_reduce(L["cc"], L["p9"], XL, AL.add)
                for L in lanes:
                    V.tensor_reduce(L["svs"], L["q9"], XL, AL.add)
                for L in lanes:
                    V.reciprocal(L["r"], L["svs"])
                for L in lanes:
                    V.tensor_tensor(L["d1"], L["cc"], L["r"], AL.mult)
                if it == 0:
                    for L in lanes:
                        V.tensor_scalar(L["lam"], L["d1"], -1.0, 0.0,
                                        AL.mult, AL.max)
                else:
                    for L in lanes:
                        V.scalar_tensor_tensor(L["d2"], L["d1"], -1.0,
                                               L["lam"], AL.mult, AL.add)
                    for L in lanes:
                        V.tensor_scalar(L["lam"], L["d2"], 0.0, None, AL.max)

            # final u = clip(lam*g - p) and store (fp32 path)
            for L in lanes:
                V.tensor_tensor(x3(L["sgx"][:]), bc(L["lam"]), x3(L["gg"]),
                                AL.mult)
            for L in lanes:
                V.tensor_tensor(L["sgx"][:], L["sgx"][:], L["p"], AL.subtract)
            for L in lanes:
                V.tensor_scalar(L["sgx"][:], L["sgx"][:], 1.0, -1.0,
                                AL.min, AL.max)
            for L in lanes:
                nc.sync.dma_start(
                    u_d[:].rearrange("(p c) j -> p c j", p=128)[:, L["ss"], :],
                    L["sgx"][:].rearrange("p (c j) -> p c j", j=8))

        for g in range(NG):
            phase_a_group(g)
            phase_b_group(g)


def _build():
    from concourse import bacc, mybir
    from concourse import tile as tile_mod
    from concourse._compat import axon_active
    f32 = mybir.dt.float32
    f32r = mybir.dt.float32r
    nc = bacc.Bacc("TRN2", target_bir_lowering=False,
                   debug=not axon_active(), num_devices=NCORES)
    x_d = nc.dram_tensor("x", [S, N], f32, kind="ExternalInput").ap()
    u_d = nc.dram_tensor("u", [S, N], f32, kind="ExternalOutput").ap()
    cds = {"CBLKr": nc.dram_tensor("CBLKr", [128, _RW], f32r,
                                   kind="ExternalInput").ap(),
           "CBLKf": nc.dram_tensor("CBLKf", [128, _FW], f32,
                                   kind="ExternalInput").ap()}
    with tile_mod.TileContext(nc) as tc:
        build_kernel(nc, tc, x_d, u_d, cds)
    nc.compile()
    return nc


def kernel(x, W1, b1, W21, b21, W22, b22, W31, b31, W32, b32, A, G, mean, std):
    from concourse.bass_utils import run_bass_kernel_spmd
    f32 = np.float32
    x = np.asarray(x, f32)
    x0 = (x * np.asarray(std, f32) + np.asarray(mean, f32)).astype(f32)

    consts = _consts(np.asarray(W1, f32), np.asarray(b1, f32), np.asarray(W21, f32),
                     np.asarray(b21, f32), np.asarray(W22, f32), np.asarray(b22, f32),
                     np.asarray(W31, f32), np.asarray(b31, f32), np.asarray(W32, f32),
                     np.asarray(b32, f32), np.asarray(A, f32), np.asarray(G, f32))
    if "nc" not in _CACHE:
        _CACHE["nc"] = _build()
    nc = _CACHE["nc"]

    cblkr = np.ascontiguousarray(np.hstack([consts[k] for k in _RKEYS]))
    cblkf = np.ascontiguousarray(np.hstack([consts[k] for k in _FKEYS]))
    in_maps = []
    for c in range(NCORES):
        m = {"x": np.ascontiguousarray(x0[c * S:(c + 1) * S]),
             "CBLKr": cblkr, "CBLKf": cblkf}
        in_maps.append(m)
    res = run_bass_kernel_spmd(nc, in_maps, list(range(NCORES)))
    out = np.concatenate([np.asarray(res.results[c]["u"]) for c in range(NCORES)],
                         axis=0)
    return out.astype(f32)


# revision 37
# speedup vs baseline: 1.0005x; 1.0005x over previous
"""Trainium2 Bass kernel for nn_BarrierPolicy (CBF-QP safety filter).

Data-parallel over batch: 8 cores x 32768 samples, processed in 4 groups of
8192 samples (4 xview tiles of 2048).

Phase A (per group): DMA x tiles, PE-transpose to SP2 layout, then the MLP
with weight-major matmul runs (one Ldweights per layer): L1 uses a single
(16,128) weight against rhs partition slices (K=16), L2 a single block-pair
weight, L3 a single fused (128,18) weight producing px and the alpha logit
together; dynamics matmuls (A x, -2 G^T x) and PE-transposes back to xview.
Activation-engine evacuations are fused into (128,1024) ops.

Phase B (per group, overlaps the next group's Phase A): optimistic-slope
Newton solve of the per-sample box-QP dual in sign-transformed space:
  ur = lam*gt - pt ; uc = clip(ur) ; c = c0 + sum(gt*uc)
  S  = sum(q * [ur < 1])   (upper bound on all future slopes -> monotone
                            convergence from below; all-saturated infeasible
                            rows diverge to huge lam = reference saturation)
  lam = max(lam - c/S, 0)
c0 and a 1e-12 epsilon ride in a 9th reduction lane. Final u = clip(lam*g-p).

Layouts (per tile of 2048 samples):
  xview: SBUF (128, 128): partition r, col 16b+8s0+j <-> sample 256b+2r+s0,
  coord j; slot: per-sample scalars (128, 16): partition r, col 2b+s0.
"""
import numpy as np

B_FULL, N = 262144, 8
NCORES = 8
S = B_FULL // NCORES          # 32768 samples per core
TILE = 2048
NT = S // TILE                # 16 tiles
NSLOT = S // 128              # 256 slot cols per core
import os
GROUPS = eval(os.environ.get('KGROUPS', '[2, 4, 4, 4, 2]'))
NLANES = int(os.environ.get('KLANES', '1'))
NG = len(GROUPS)              # group for early B start, small last for tail
GBASE = [sum(GROUPS[:i]) for i in range(NG)]
GROUP = 4                     # max group size (psum/scratch tile shapes)
FCG = 128 * GROUP
SLG = 16 * GROUP
T_NEWTON = 5
EPS = 1e-12

_CACHE = {}

_CSHAPES = dict(TL2=(128, 128), TL3F=(128, 18),
                TDA=(128, 128), TDG=(128, 128), ID128=(128, 128), IDr=(128, 128),
                B1v=(128, 1), B2v=(128, 1), B31x=(128, 1), B32e=(128, 1),
                **{f"TL1E{b}": (128, 128) for b in range(8)})
_RKEYS = ["TL2", "TDA", "TDG", "IDr"] + [f"TL1E{b}" for b in range(8)]
_FKEYS = ["TL3F", "ID128", "B1v", "B2v", "B31x", "B32e"]
_RW = sum(_CSHAPES[k][1] for k in _RKEYS)
_FW = sum(_CSHAPES[k][1] for k in _FKEYS)


def _consts(W1, b1, W21, b21, W22, b22, W31, b31, W32, b32, A, G):
    f32 = np.float32
    out = {}
    for b in range(8):
        T = np.zeros((128, 128), f32)
        for s0 in range(2):
            T[16 * b + 8 * s0:16 * b + 8 * s0 + 8, 64 * s0:64 * s0 + 64] = W1
        out[f"TL1E{b}"] = T
    TL2 = np.zeros((128, 128), f32)
    for s0 in range(2):
        TL2[64 * s0:64 * s0 + 64, 32 * s0:32 * s0 + 32] = W21
        TL2[64 * s0:64 * s0 + 64, 64 + 32 * s0:64 + 32 * s0 + 32] = W22
    TL3F = np.zeros((128, 18), f32)        # fused px + alpha-logit head
    for s0 in range(2):
        TL3F[32 * s0:32 * s0 + 32, 8 * s0:8 * s0 + 8] = W31
        TL3F[64 + 32 * s0:96 + 32 * s0, 16 + s0:17 + s0] = W32
    TDA = np.kron(np.eye(16, dtype=f32), A.T.astype(f32))         # out = A x
    TDG = np.kron(np.eye(16, dtype=f32), (-2.0 * G).astype(f32))  # out = -2 G^T x
    ID128 = np.eye(128, dtype=f32)
    B1v = np.concatenate([b1, b1]).reshape(128, 1).astype(f32)
    B2v = np.concatenate([b21, b21, b22, b22]).reshape(128, 1).astype(f32)
    B31x = np.zeros((128, 1), f32)         # px bias rows; alpha rows stay 0
    for m in range(4):
        for s0 in range(2):
            B31x[32 * m + 8 * s0:32 * m + 8 * s0 + 8, 0] = b31
    B32e = np.full((128, 1), float(b32[0]), f32)
    out.update(TL2=TL2, TL3F=TL3F, TDA=TDA, TDG=TDG, ID128=ID128, IDr=ID128,
               B1v=B1v, B2v=B2v, B31x=B31x, B32e=B32e)
    return out


def build_kernel(nc, tc, x_d, u_d, cds):
    from concourse import mybir
    f32 = mybir.dt.float32
    f32r = mybir.dt.float32r
    bf16 = mybir.dt.bfloat16
    AL = mybir.AluOpType
    AF = mybir.ActivationFunctionType
    XL = mybir.AxisListType.X

    with (
        tc.tile_pool(name="const", bufs=1) as cpool,
        tc.tile_pool(name="pers", bufs=1) as pers,
        tc.tile_pool(name="work", bufs=2) as work,
        tc.tile_pool(name="psT", bufs=2, space="PSUM") as psT,
        tc.tile_pool(name="psW", bufs=2, space="PSUM") as psW,
        tc.tile_pool(name="psL", bufs=1, space="PSUM") as psL,
    ):
        CBR = cpool.tile([128, _RW], f32r, tag="CBR", name="CBR")
        CBF = cpool.tile([128, _FW], f32, tag="CBF", name="CBF")
        nc.sync.dma_start(CBR[:], cds["CBLKr"][:])
        nc.sync.dma_start(CBF[:], cds["CBLKf"][:])
        C, off = {}, 0
        for k in _RKEYS:
            w = _CSHAPES[k][1]
            C[k] = CBR[:, off:off + w]
            off += w
        off = 0
        for k in _FKEYS:
            w = _CSHAPES[k][1]
            C[k] = CBF[:, off:off + w]
            off += w

        FC = S // 16   # 2048 xview cols per core
        def fc_tile(tag):
            return pers.tile([128, FC], f32, tag=tag, name=tag)
        x_xv, p_xv, g_xv = fc_tile("x_xv"), fc_tile("p_xv"), fc_tile("g_xv")
        gt_xv, q_xv = fc_tile("gt_xv"), fc_tile("q_xv")
        pt_xv = pers.tile([128, FC], bf16, tag="pt_xv", name="pt_xv")
        def sl_tile(tag, mult=1):
            return pers.tile([128, NSLOT * mult], f32, tag=tag, name=tag)
        alpha4, lfhx, sxx = sl_tile("alpha4"), sl_tile("lfhx"), sl_tile("sxx")
        lams, ccs, sss = sl_tile("lams"), sl_tile("ccs"), sl_tile("sss")
        rs, t1s, t2s = sl_tile("rs"), sl_tile("t1s"), sl_tile("t2s")
        prod9, qm9 = sl_tile("prod9", 9), sl_tile("qm9", 9)
        nc.gpsimd.memset(
            qm9[:].rearrange("p (c j) -> p c j", j=9)[:, :, 8:9], EPS)

        x3 = lambda ap: ap.rearrange("p (c j) -> p c j", j=8)
        x9 = lambda ap: ap.rearrange("p (c j) -> p c j", j=9)
        V, GP, SC = nc.vector, nc.gpsimd, nc.scalar

        def phase_a_group(g):
            nt = GROUPS[g]
            tb = GBASE[g]
            fcg, slg = 128 * nt, 16 * nt
            csg = slice(128 * tb, 128 * (tb + nt))
            ssg = slice(16 * tb, 16 * (tb + nt))
            # ---- load + transpose to SP2 ----
            nc.sync.dma_start(
                x_xv[:, csg].rearrange("p (c j) -> p c j", j=8),
                x_d[:].rearrange("(p c) j -> p c j", p=128)[:, ssg, :])
            xTP = psT.tile([128, 4, 128], f32, tag="tp", name="xTP")
            for tt in range(nt):
                t = tb + tt
                cs = slice(128 * t, 128 * t + 128)
                nc.tensor.transpose(xTP[:, tt, :], x_xv[:, cs], C["ID128"])
            xsp2g = work.tile([128, 4, 128], f32r, tag="xsp2g", name="xsp2g")
            SC.activation(xsp2g[:, 0:nt, :], xTP[:, 0:nt, :], AF.Copy)

            # ---- L1 (weight-major, free=512) ----
            h1 = work.tile([128, 8, 512], f32r, tag="h1", name="h1", bufs=2)
            for pair in range(4):
                h1P = psW.tile([128, 2, 512], f32, tag="mmP", name="h1P")
                for s in range(2):
                    b = 2 * pair + s
                    nc.tensor.matmul(h1P[:, s, 0:fcg], C[f"TL1E{b}"],
                                     xsp2g[:, 0:nt, :])
                SC.activation(h1[:, 2 * pair:2 * pair + 2, 0:fcg],
                              h1P[:, :, 0:fcg], AF.Relu, bias=C["B1v"])

            # ---- L2 (single weight) ----
            x2 = work.tile([128, 8, 512], f32, tag="x2", name="x2", bufs=2)
            for pair in range(4):
                x2P = psW.tile([128, 2, 512], f32, tag="mmP", name="x2P")
                for s in range(2):
                    b = 2 * pair + s
                    nc.tensor.matmul(x2P[:, s, 0:fcg], C["TL2"],
                                     h1[:, b, 0:fcg])
                SC.activation(x2[:, 2 * pair:2 * pair + 2, 0:fcg],
                              x2P[:, :, 0:fcg], AF.Relu, bias=C["B2v"])

            # ---- L3 fused px+alpha (single (128,18) weight) ----
            pxale = work.tile([128, 3, 512], f32r, tag="pxale", name="pxale",
                              bufs=2)
            for hf in range((fcg + 255) // 256):
                w = min(256, fcg - 256 * hf)
                fs = slice(256 * hf, 256 * hf + w)
                pxalP = psL.tile([128, 3, 256], f32, tag="pxalP",
                                 name=f"pxalP{hf}")
                for b in range(8):
                    m3, k3 = b % 3, b // 3
                    nc.tensor.matmul(pxalP[32 * m3:32 * m3 + 18, k3, 0:w],
                                     C["TL3F"], x2[:, b, fs])
                for m3 in range(3):
                    kk = 3 if m3 < 2 else 2
                    SC.activation(pxale[32 * m3:32 * m3 + 18, 0:kk, fs],
                                  pxalP[32 * m3:32 * m3 + 18, 0:kk, 0:w],
                                  AF.Identity,
                                  bias=C["B31x"][32 * m3:32 * m3 + 18, :])

            # ---- dynamics ----
            dynA = psT.tile([128, 4, 128], f32, tag="tp", name="dynA")
            nc.tensor.matmul(dynA[:, 0:nt, :], C["TDA"], xsp2g[:, 0:nt, :])
            axsg = work.tile([128, 4, 128], f32r, tag="axsg", name="axsg")
            SC.activation(axsg[:, 0:nt, :], dynA[:, 0:nt, :], AF.Copy)
            dynG = psT.tile([128, 4, 128], f32, tag="tp", name="dynG")
            nc.tensor.matmul(dynG[:, 0:nt, :], C["TDG"], xsp2g[:, 0:nt, :])
            gsp2g = work.tile([128, 4, 128], f32r, tag="gsp2g", name="gsp2g")
            SC.activation(gsp2g[:, 0:nt, :], dynG[:, 0:nt, :], AF.Copy)

            # ---- transposes back to xview ----
            gT = psT.tile([128, 4, 128], f32r, tag="tp", name="gT")
            for tt in range(nt):
                nc.tensor.transpose(gT[:, tt, :], gsp2g[:, tt, :], C["IDr"])
            SC.activation(g_xv[:, csg].rearrange("p (a b) -> p a b", a=nt),
                          gT[:, 0:nt, :], AF.Copy)
            aT = psT.tile([128, 4, 128], f32r, tag="tp", name="aT")
            for tt in range(nt):
                nc.tensor.transpose(aT[:, tt, :], axsg[:, tt, :], C["IDr"])

            # ---- barrier scalars: Lfhx, ||x||^2 ----
            prodA = work.tile([128, 4, 128], f32, tag="prodA", name="prodA")
            V.scalar_tensor_tensor(
                prodA[:, 0:nt, :], aT[:, 0:nt, :], -2.0,
                x_xv[:, csg].rearrange("p (a b) -> p a b", a=nt),
                AL.mult, AL.mult)
            V.tensor_reduce(lfhx[:, ssg],
                            prodA[:, 0:nt, :].rearrange(
                                "p a (c j) -> p (a c) j", j=8),
                            XL, AL.add)
            sqxg = work.tile([128, 512], f32, tag="sqxg", name="sqxg")
            GP.tensor_tensor(sqxg[:, 0:fcg], x_xv[:, csg], x_xv[:, csg],
                             AL.mult)
            V.tensor_reduce(sxx[:, ssg], x3(sqxg[:, 0:fcg]), XL, AL.add)

            # ---- px/alpha transposes + extraction ----
            for k3 in range(3):
                nm = 3 if k3 < 2 else 2
                pT = psT.tile([128, 4, 128], f32r, tag="tp", name=f"pT{k3}")
                for tt in range(nt):
                    nc.tensor.transpose(pT[:, tt, :],
                                        pxale[:, k3, 128 * tt:128 * tt + 128],
                                        C["IDr"])
                src = pT[:, 0:nt, :].rearrange("p t (m h x) -> p t m h x",
                                               m=4, h=2, x=16)
                dstp = p_xv[:, csg].rearrange(
                    "p (t b x) -> p t b x", t=nt, b=8,
                    x=16)[:, :, 3 * k3:3 * k3 + nm]
                V.tensor_copy(dstp, src[:, :, 0:nm, 0, :])
                dsta = alpha4[:, ssg].rearrange(
                    "p (t b s) -> p t b s", t=nt, b=8,
                    s=2)[:, :, 3 * k3:3 * k3 + nm]
                V.tensor_copy(dsta, src[:, :, 0:nm, 1, 0:2])
            SC.activation(alpha4[:, ssg], alpha4[:, ssg], AF.Sigmoid,
                          bias=C["B32e"])

        def phase_b_group(g, nlanes=NLANES):
            nt = GROUPS[g]
            tb = GBASE[g]
            fcg, slg = 128 * nt // nlanes, 16 * nt // nlanes
            lanes = []
            for h in range(nlanes):
                cs = slice(128 * tb + fcg * h, 128 * tb + fcg * (h + 1))
                ss = slice(16 * tb + slg * h, 16 * tb + slg * (h + 1))
                s9 = slice(9 * (16 * tb + slg * h),
                           9 * (16 * tb + slg * (h + 1)))
                L = dict(
                    gt=gt_xv[:, cs], pt=pt_xv[:, cs], q=q_xv[:, cs],
                    p=p_xv[:, cs], gg=g_xv[:, cs],
                    lam=lams[:, ss], cc=ccs[:, ss], svs=sss[:, ss],
                    r=rs[:, ss], d1=t1s[:, ss], d2=t2s[:, ss],
                    p9=x9(prod9[:, s9]), q9=x9(qm9[:, s9]),
                    sxx=sxx[:, ss], alpha4=alpha4[:, ss], lfhx=lfhx[:, ss],
                    ss=ss,
                    sgx=work.tile([128, fcg], f32, tag=f"sgx{h}", name="sgx"),
                    ur=work.tile([128, fcg], bf16, tag=f"ur{h}", name="ur"),
                    uc=work.tile([128, fcg], bf16, tag=f"uc{h}", name="uc"),
                    mt=work.tile([128, fcg], bf16, tag=f"mt{h}", name="mt"),
                )
                lanes.append(L)
            bc = lambda ap: ap.broadcast_to((128, slg, 8))

            # preamble: transform + c0 (c0 lands in prod9's 9th lane)
            for L in lanes:
                SC.sign(L["sgx"][:], L["gg"])
            for L in lanes:
                SC.activation(L["gt"], L["gg"], AF.Abs)
            for L in lanes:
                GP.tensor_tensor(L["q"], L["gg"], L["gg"], AL.mult)
            for L in lanes:
                V.tensor_tensor(L["pt"], L["sgx"][:], L["p"], AL.mult)
            for L in lanes:
                GP.tensor_scalar(L["d1"], L["sxx"], -4.0, 64.0, AL.mult, AL.add)
            for L in lanes:
                GP.tensor_tensor(L["d2"], L["alpha4"], L["d1"], AL.mult)
            for L in lanes:
                GP.tensor_tensor(L["p9"][:, :, 8], L["d2"], L["lfhx"], AL.add)

            for it in range(T_NEWTON):
                if it == 0:
                    # lam = 0: ur = -pt
                    for L in lanes:
                        V.tensor_scalar(L["uc"][:], L["pt"], -1.0, 1.0,
                                        AL.mult, AL.min)
                    for L in lanes:
                        V.tensor_scalar(L["uc"][:], L["uc"][:], -1.0, None,
                                        AL.max)
                    for L in lanes:
                        GP.tensor_tensor(L["p9"][:, :, 0:8], x3(L["gt"]),
                                         x3(L["uc"][:]), AL.mult)
                    for L in lanes:
                        V.tensor_scalar(L["mt"][:], L["pt"], -1.0, None,
                                        AL.is_gt)
                else:
                    for L in lanes:
                        GP.tensor_tensor(x3(L["ur"][:]), bc(L["lam"]),
                                         x3(L["gt"]), AL.mult)
                    for L in lanes:
                        V.tensor_tensor(L["ur"][:], L["ur"][:], L["pt"],
                                        AL.subtract)
                    for L in lanes:
                        V.tensor_scalar(L["uc"][:], L["ur"][:], 1.0, -1.0,
                                        AL.min, AL.max)
                    for L in lanes:
                        GP.tensor_tensor(L["p9"][:, :, 0:8], x3(L["gt"]),
                                         x3(L["uc"][:]), AL.mult)
                    for L in lanes:
                        V.tensor_scalar(L["mt"][:], L["ur"][:], 1.0, None,
                                        AL.is_lt)
                for L in lanes:
                    GP.tensor_tensor(L["q9"][:, :, 0:8], x3(L["q"]),
                                     x3(L["mt"][:]), AL.mult)
                for L in lanes:
                    V.tensor_reduce(L["cc"], L["p9"], XL, AL.add)
                for L in lanes:
                    V.tensor_reduce(L["svs"], L["q9"], XL, AL.add)
                for L in lanes:
                    V.reciprocal(L["r"], L["svs"])
                for L in lanes:
                    V.tensor_tensor(L["d1"], L["cc"], L["r"], AL.mult)
                if it == 0:
                    for L in lanes:
                        V.tensor_scalar(L["lam"], L["d1"], -1.0, 0.0,
                                        AL.mult, AL.max)
                else:
                    for L in lanes:
                        V.scalar_tensor_tensor(L["d2"], L["d1"], -1.0,
                                               L["lam"], AL.mult, AL.add)
                    for L in lanes:
                        V.tensor_scalar(L["lam"], L["d2"], 0.0, None, AL.max)

            # final u = clip(lam*g - p) and store (fp32 path)
            for L in lanes:
                V.tensor_tensor(x3(L["sgx"][:]), bc(L["lam"]), x3(L["gg"]),
                                AL.mult)
            for L in lanes:
                V.tensor_tensor(L["sgx"][:], L["sgx"][:], L["p"], AL.subtract)
            for L in lanes:
                V.tensor_scalar(L["sgx"][:], L["sgx"][:], 1.0, -1.0,
                                AL.min, AL.max)
            for L in lanes:
                nc.sync.dma_start(
                    u_d[:].rearrange("(p c) j -> p c j", p=128)[:, L["ss"], :],
                    L["sgx"][:].rearrange("p (c j) -> p c j", j=8))

        for g in range(NG):
            phase_a_group(g)
            phase_b_group(g)


def _build():
    from concourse import bacc, mybir
    from concourse import tile as tile_mod
    from concourse._compat import axon_active
    f32 = mybir.dt.float32
    f32r = mybir.dt.float32r
    nc = bacc.Bacc("TRN2", target_bir_lowering=False,
                   debug=not axon_active(), num_devices=NCORES)
    x_d = nc.dram_tensor("x", [S, N], f32, kind="ExternalInput").ap()
    u_d = nc.dram_tensor("u", [S, N], f32, kind="ExternalOutput").ap()
    cds = {"CBLKr": nc.dram_tensor("CBLKr", [128, _RW], f32r,
                                   kind="ExternalInput").ap(),
           "CBLKf": nc.dram_tensor("CBLKf", [128, _FW], f32,
                                   kind="ExternalInput").ap()}
    with tile_mod.TileContext(nc) as tc:
        build_kernel(nc, tc, x_d, u_d, cds)
    nc.compile()
    return nc


def kernel(x, W1, b1, W21, b21, W22, b22, W31, b31, W32, b32, A, G, mean, std):
    from concourse.bass_utils import run_bass_kernel_spmd
    f32 = np.float32
    x = np.asarray(x, f32)
    x0 = (x * np.asarray(std, f32) + np.asarray(mean, f32)).astype(f32)

    consts = _consts(np.asarray(W1, f32), np.asarray(b1, f32), np.asarray(W21, f32),
                     np.asarray(b21, f32), np.asarray(W22, f32), np.asarray(b22, f32),
                     np.asarray(W31, f32), np.asarray(b31, f32), np.asarray(W32, f32),
                     np.asarray(b32, f32), np.asarray(A, f32), np.asarray(G, f32))
    if "nc" not in _CACHE:
        _CACHE["nc"] = _build()
    nc = _CACHE["nc"]

    cblkr = np.ascontiguousarray(np.hstack([consts[k] for k in _RKEYS]))
    cblkf = np.ascontiguousarray(np.hstack([consts[k] for k in _FKEYS]))
    in_maps = []
    for c in range(NCORES):
        m = {"x": np.ascontiguousarray(x0[c * S:(c + 1) * S]),
             "CBLKr": cblkr, "CBLKf": cblkf}
        in_maps.append(m)
    res = run_bass_kernel_spmd(nc, in_maps, list(range(NCORES)))
    out = np.concatenate([np.asarray(res.results[c]["u"]) for c in range(NCORES)],
                         axis=0)
    return out.astype(f32)
